# revision 12
# baseline (speedup 1.0000x reference)
"""BiRGAT (2-layer heterogeneous GATv2) on 8 Trainium2 NeuronCores.

Sharding:
  - sg relation (mrna->gene): edges dst-sharded by gene range (6250/core);
    segment softmax is core-local.
  - gs relation (gene->mrna) and layer-3: edges src-sharded by the same gene
    ranges; each core produces partial per-dst (mrna) weighted sums + softmax
    denominators which the host reduces (the unshard step).
Two launches: L1 (both relations) -> host combines x1 -> L2 (layer 3).

Device pipeline per 128-edge group (edges on partitions, features on free):
  dma_gather xl[src], xr[dst] rows (bf16 tables in DRAM)
  PSUM s = ea (x) We + I@xl + I@xr   (3 matmuls)
  m = LeakyRelu(s) [ACT], logits = reduce(m*att) [DVE], ex = Exp(logits) [ACT]
  aug = [xl * ex_bcast | ex]  (DVE)
  window matmul: psum_w += sel^T @ aug, sel = is_equal(dst_rel, iota)
  (windows = 127 consecutive dst ids + 1 pad slot; edge list padded per
   window to G*128 slots so the whole schedule is static)
Softmax uses exp without max subtraction (logits are provably tiny here);
output bias bo is folded into the xl table (+bo) and xr table (-bo) so the
alpha-weighted sum directly yields out + bo (softmax weights sum to 1).
"""

import math
from dataclasses import dataclass, field

import ml_dtypes
import numpy as np

import concourse.bacc as bacc
import concourse.bass as bass
import concourse.mybir as mybir
import concourse.tile as tile
from concourse import library_config
from concourse.bass_utils import run_bass_kernel_spmd

F32 = mybir.dt.float32
BF16 = mybir.dt.bfloat16
I16 = mybir.dt.int16
NEG_SLOPE = 0.2
WIN = 127  # real dst ids per window (slot 127 = pad)
CH = 4096  # edges per gather chunk

bf = lambda a: np.ascontiguousarray(a).astype(ml_dtypes.bfloat16)
f32 = lambda a: np.ascontiguousarray(a).astype(np.float32)


@dataclass
class Cfg:
    n_s: int = 4000
    n_g: int = 50000
    e: int = 250000
    d: int = 128
    h: int = 4
    c1: int = 32
    c3: int = 64
    ncores: int = 8

    @property
    def hc1(self):
        return self.h * self.c1

    @property
    def hc3(self):
        return self.h * self.c3

    @property
    def gsh(self):
        return self.n_g // self.ncores


def _wrap_idx(idx):
    """[n] -> [128, n//16] int16, wrapped in 16 partitions, replicated x8."""
    n = len(idx)
    w = idx.reshape(n // 16, 16).T.astype(np.int16)
    return np.ascontiguousarray(np.tile(w, (8, 1)))


def _pm(v, n):
    """[n] -> [128, n//128] partition-major gather layout."""
    return np.ascontiguousarray(v.reshape(n // 128, 128).T)


def _pack_edges(srcl, dstl, ea, w_count, g):
    """Sort by dst, lay out into per-window slabs of g*128 slots."""
    n = w_count * g * 128
    order = np.argsort(dstl, kind="stable")
    s, d, e = srcl[order], dstl[order], ea[order]
    w = d // WIN
    cnt = np.bincount(w, minlength=w_count)
    assert cnt.max() <= g * 128, (cnt.max(), g * 128)
    start = np.zeros(w_count, np.int64)
    start[1:] = np.cumsum(cnt)[:-1]
    slot = w * g * 128 + (np.arange(len(d)) - start[w])
    S = np.zeros(n, np.int64)
    D = np.zeros(n, np.int64)
    Dr = np.full(n, WIN, np.int64)
    Ea = np.zeros(n, np.float32)
    S[slot] = s
    D[slot] = d
    Dr[slot] = d - WIN * w
    Ea[slot] = e
    return S, D, Dr, Ea


def _edge_inputs(prefix, S, D, Dr, Ea, n):
    return {
        f"{prefix}_src": _wrap_idx(S),
        f"{prefix}_dst": _wrap_idx(D),
        f"{prefix}_dr": bf(_pm(Dr, n)),
        f"{prefix}_ea": bf(Ea.reshape(1, n)),
    }


def _emit_table(nc, pools, xT_sb, w_sb, b_sb, ones_sb, table, v, fdim):
    """table[v, fdim] (bf16 DRAM) = xT.T @ w + b, chunked by 128 rows."""
    sb_pool, ps_pool = pools
    for i in range(math.ceil(v / 128)):
        m = min(128, v - i * 128)
        ps = ps_pool.tile([128, fdim], F32, tag="tfps")
        nc.tensor.matmul(
            out=ps[:m], lhsT=xT_sb[:, i * 128 : i * 128 + m], rhs=w_sb[:],
            start=True, stop=False,
        )
        nc.tensor.matmul(
            out=ps[:m], lhsT=ones_sb[0:1, :m], rhs=b_sb[:], start=False, stop=True,
        )
        sb = sb_pool.tile([128, fdim], BF16, tag="tfsb")
        nc.scalar.copy(sb[:m], ps[:m])
        nc.sync.dma_start(out=table[i * 128 : i * 128 + m, :], in_=sb[:m])


def _emit_pass(nc, tc, pools, consts, *, n, w_count, g, fdim, h, xl_t, xr_t,
               src_d, dst_d, ea_d, dr_d, we_sb, att_sb, slab, vd, level=3):
    """One GAT edge pass."""
    sb_pool, ps_pool, gat_pool, win_pool = pools
    ident_sb, iota_sb = consts
    c = fdim // h
    bgrp = 512 // fdim  # groups per s-psum bank
    fa = fdim + 4  # aug width (features + per-head ex)

    psw = None
    for c0 in range(0, n, CH):
        csz = min(CH, n - c0)
        ngr = csz // 128
        xi = gat_pool.tile([128, csz // 16], I16, tag="xi")
        ri = gat_pool.tile([128, csz // 16], I16, tag="ri")
        nc.sync.dma_start(out=xi[:], in_=src_d[:, c0 // 16 : (c0 + csz) // 16])
        nc.sync.dma_start(out=ri[:], in_=dst_d[:, c0 // 16 : (c0 + csz) // 16])
        xlg = gat_pool.tile([128, ngr, fdim], BF16, tag="xlg")
        xrg = gat_pool.tile([128, ngr, fdim], BF16, tag="xrg")
        nc.gpsimd.dma_gather(xlg[:], xl_t[:], xi[:], csz, csz, fdim, single_packet=False)
        nc.gpsimd.dma_gather(xrg[:], xr_t[:], ri[:], csz, csz, fdim, single_packet=False)
        ea_sb = gat_pool.tile([1, csz], BF16, tag="ea")
        dr_sb = gat_pool.tile([128, ngr], BF16, tag="dr")
        nc.sync.dma_start(out=ea_sb[:], in_=ea_d[:, c0 : c0 + csz])
        nc.sync.dma_start(out=dr_sb[:], in_=dr_d[:, c0 // 128 : c0 // 128 + ngr])

        if level == 0:
            if c0 == 0:
                dbg = gat_pool.tile([128, 512], F32, tag="dbg")
                nc.vector.tensor_copy(out=dbg[:, :fdim], in_=xlg[:, 0, :])
                nc.vector.tensor_copy(out=dbg[:, fdim : fdim + 4], in_=xrg[:, 0, 0:4])
                nc.sync.dma_start(out=slab[0:128, :], in_=dbg[:128, :fa])
            continue
        for b0 in range(0, ngr, bgrp):
            nb = min(bgrp, ngr - b0)
            sps = ps_pool.tile([128, 512], F32, tag="sps")
            for j in range(b0, b0 + nb):
                col = (j - b0) * fdim
                o = sps[:, col : col + fdim]
                nc.tensor.matmul(
                    out=o, lhsT=ea_sb[0:1, j * 128 : (j + 1) * 128],
                    rhs=we_sb[:], start=True, stop=False,
                )
                nc.tensor.matmul(
                    out=o, lhsT=ident_sb[:], rhs=xlg[:, j, :], start=False,
                    stop=False,
                )
                nc.tensor.matmul(
                    out=o, lhsT=ident_sb[:], rhs=xrg[:, j, :], start=False,
                    stop=True,
                )
            nf = nb * fdim
            r_sb = gat_pool.tile([128, 512], BF16, tag="r")
            nc.scalar.mul(r_sb[:, :nf], sps[:, :nf], NEG_SLOPE)
            m_sb = gat_pool.tile([128, 512], BF16, tag="m")
            nc.vector.tensor_tensor(
                out=m_sb[:, :nf], in0=sps[:, :nf], in1=r_sb[:, :nf],
                op=mybir.AluOpType.max,
            )
            am = gat_pool.tile([128, 512], BF16, tag="am")
            nc.vector.tensor_tensor(
                out=am[:, :nf], in0=m_sb[:, :nf],
                in1=att_sb[:, :nf],
                op=mybir.AluOpType.mult,
            )
            lgt = gat_pool.tile([128, bgrp * h], F32, tag="lgt")
            nc.vector.tensor_reduce(
                out=lgt[:, : nb * h],
                in_=am[:, :nf].rearrange("p (x c) -> p x c", c=c),
                axis=mybir.AxisListType.X, op=mybir.AluOpType.add,
            )
            if level == 1:
                dbg = gat_pool.tile([128, 512], F32, tag="dbg")
                nc.scalar.copy(dbg[:, :nf], m_sb[:, :nf])
                if b0 == 0 and c0 == 0:
                    nc.sync.dma_start(out=slab[0:128, :], in_=dbg[:128, :fa])
                continue
            ex = gat_pool.tile([128, bgrp * h], BF16, tag="ex")
            nc.scalar.activation(
                ex[:, : nb * h], lgt[:, : nb * h],
                mybir.ActivationFunctionType.Exp,
            )
            aug = gat_pool.tile([128, bgrp * fa], BF16, tag="aug")
            aug3 = aug[:].rearrange("p (b x) -> p b x", b=bgrp)
            nc.vector.tensor_tensor(
                out=aug3[:, :nb, :fdim].rearrange("p b (hh cc) -> p b hh cc", cc=c),
                in0=xlg[:, b0 : b0 + nb, :].rearrange(
                    "p b (hh cc) -> p b hh cc", cc=c
                ),
                in1=ex[:, : nb * h].rearrange("p (b hh) -> p b hh", hh=h)
                .unsqueeze(3).to_broadcast([128, nb, h, c]),
                op=mybir.AluOpType.mult,
            )
            nc.vector.tensor_copy(
                out=aug3[:, :nb, fdim:fa],
                in_=ex[:, : nb * h].rearrange("p (b hh) -> p b hh", hh=h),
            )
            if level == 2:
                dbg = gat_pool.tile([128, 512], F32, tag="dbg")
                nc.scalar.copy(dbg[:, : nb * fa], aug[:, : nb * fa])
                if b0 == 0 and c0 == 0:
                    nc.sync.dma_start(out=slab[0:128, :], in_=dbg[:128, :fa])
                continue
            for j in range(b0, b0 + nb):
                grp = c0 // 128 + j
                wi, jw = divmod(grp, g)
                if jw == 0:
                    psw = win_pool.tile([128, fa], F32, tag="psw")
                sel = gat_pool.tile([128, 128], BF16, tag="sel")
                nc.vector.tensor_tensor(
                    out=sel[:],
                    in0=dr_sb[:, j : j + 1].to_broadcast([128, 128]),
                    in1=iota_sb[:],
                    op=mybir.AluOpType.is_equal,
                )
                nc.tensor.matmul(
                    out=psw[:],
                    lhsT=sel[:],
                    rhs=aug[:, (j - b0) * fa : (j - b0 + 1) * fa],
                    start=(jw == 0), stop=(jw == g - 1),
                )
                if jw == g - 1:
                    rows = min(WIN, vd - WIN * wi)
                    osb = gat_pool.tile([128, fa], F32, tag="osb")
                    nc.scalar.copy(osb[:], psw[:])
                    nc.sync.dma_start(
                        out=slab[WIN * wi : WIN * wi + rows, :], in_=osb[:rows],
                    )


def _emit_sl(nc, pools, wsl_sb, xT_sb, out_d, cx, v):
    """out_d[cx, v] f32 = wsl.T @ x  (self-loop linear), chunked by 512."""
    sb_pool, ps_pool = pools
    for i in range(math.ceil(v / 512)):
        m = min(512, v - i * 512)
        ps = ps_pool.tile([cx, 512], F32, tag="slps")
        nc.tensor.matmul(
            out=ps[:, :m], lhsT=wsl_sb[:], rhs=xT_sb[:, i * 512 : i * 512 + m],
            start=True, stop=True,
        )
        sb = sb_pool.tile([cx, 512], F32, tag="slsb")
        nc.scalar.copy(sb[:, :m], ps[:, :m])
        nc.sync.dma_start(out=out_d[:, i * 512 : i * 512 + m], in_=sb[:, :m])


def _load_sb(nc, pool, name, arr_shape, dtype, dram):
    t = pool.tile(list(arr_shape), dtype, tag=name)
    nc.sync.dma_start(out=t[:], in_=dram[:])
    return t


def _build_launch1(cfg: Cfg, g_sg, g_gs, w_sg, w_gs, parts=("tables", "sl", "sg", "gs")):
    n_sg = w_sg * g_sg * 128
    n_gs = w_gs * g_gs * 128
    hc1 = cfg.hc1
    nc = bacc.Bacc("TRN2", target_bir_lowering=False, debug=False,
                   num_devices=cfg.ncores)
    dI, dO = "ExternalInput", "ExternalOutput"
    D = {}
    def di(name, shape, dt):
        D[name] = nc.dram_tensor(name, list(shape), dt, kind=dI)
        return D[name]

    di("xmT", (128, cfg.n_s), BF16)
    di("xgT", (128, cfg.gsh), BF16)
    di("ident", (128, 128), BF16)
    di("iota", (128, 128), BF16)
    di("ones", (1, 128), BF16)
    for r in ("sg", "gs"):
        di(f"w_{r}_l", (cfg.d, hc1), BF16)
        di(f"w_{r}_r", (cfg.d, hc1), BF16)
        di(f"b_{r}_l", (1, hc1), BF16)
        di(f"b_{r}_r", (1, hc1), BF16)
        di(f"we_{r}", (1, hc1), BF16)
        di(f"att_{r}", (128, 512), BF16)
    di("wsl1", (cfg.d, cfg.c1), BF16)
    for r, n in (("sg", n_sg), ("gs", n_gs)):
        di(f"{r}_src", (128, n // 16), I16)
        di(f"{r}_dst", (128, n // 16), I16)
        di(f"{r}_dr", (128, n // 128), BF16)
        di(f"{r}_ea", (1, n), BF16)

    sg_slab = nc.dram_tensor("sg_slab", [cfg.gsh, hc1 + 4], F32, kind=dO)
    gs_slab = nc.dram_tensor("gs_slab", [cfg.n_s, hc1 + 4], F32, kind=dO)
    sl1 = nc.dram_tensor("sl1", [cfg.c1, cfg.n_s], F32, kind=dO)

    t_sg_xl = nc.dram_tensor("t_sg_xl", [cfg.n_s, hc1], BF16)
    t_sg_xr = nc.dram_tensor("t_sg_xr", [cfg.gsh, hc1], BF16)
    t_gs_xl = nc.dram_tensor("t_gs_xl", [cfg.gsh, hc1], BF16)
    t_gs_xr = nc.dram_tensor("t_gs_xr", [cfg.n_s, hc1], BF16)

    with tile.TileContext(nc) as tc:
        import contextlib
        with contextlib.ExitStack() as ctx:
            persist = ctx.enter_context(tc.tile_pool(name="persist", bufs=1))
            sb_pool = ctx.enter_context(tc.tile_pool(name="sb", bufs=3))
            ps_pool = ctx.enter_context(
                tc.tile_pool(name="ps", bufs=2, space="PSUM"))
            win_pool = ctx.enter_context(
                tc.tile_pool(name="win", bufs=2, space="PSUM"))
            gat_pool = ctx.enter_context(tc.tile_pool(name="gat", bufs=2))

            nc.gpsimd.load_library(library_config.mlp)

            S = {}
            for name in ["xmT", "xgT", "ident", "iota", "ones", "wsl1",
                         "w_sg_l", "w_sg_r", "b_sg_l", "b_sg_r", "we_sg",
                         "att_sg", "w_gs_l", "w_gs_r", "b_gs_l", "b_gs_r",
                         "we_gs", "att_gs"]:
                d = D[name]
                S[name] = _load_sb(nc, persist, name, d.shape, d.dtype, d)

            tp = (sb_pool, ps_pool)
            if "tables" in parts:
                _emit_table(nc, tp, S["xmT"], S["w_sg_l"], S["b_sg_l"], S["ones"],
                            t_sg_xl, cfg.n_s, hc1)
                _emit_table(nc, tp, S["xgT"], S["w_sg_r"], S["b_sg_r"], S["ones"],
                            t_sg_xr, cfg.gsh, hc1)
                _emit_table(nc, tp, S["xgT"], S["w_gs_l"], S["b_gs_l"], S["ones"],
                            t_gs_xl, cfg.gsh, hc1)
                _emit_table(nc, tp, S["xmT"], S["w_gs_r"], S["b_gs_r"], S["ones"],
                            t_gs_xr, cfg.n_s, hc1)
            if "sl" in parts:
                _emit_sl(nc, tp, S["wsl1"], S["xmT"], sl1, cfg.c1, cfg.n_s)

            pools = (sb_pool, ps_pool, gat_pool, win_pool)
            consts = (S["ident"], S["iota"])
            if "sg" in parts:
                _emit_pass(nc, tc, pools, consts, n=n_sg, w_count=w_sg, g=g_sg,
                           fdim=hc1, h=cfg.h, xl_t=t_sg_xl, xr_t=t_sg_xr,
                           src_d=D["sg_src"], dst_d=D["sg_dst"], ea_d=D["sg_ea"],
                           dr_d=D["sg_dr"], we_sb=S["we_sg"], att_sb=S["att_sg"],
                           slab=sg_slab, vd=cfg.gsh)
            if "gs" in parts:
                _emit_pass(nc, tc, pools, consts, n=n_gs, w_count=w_gs, g=g_gs,
                           fdim=hc1, h=cfg.h, xl_t=t_gs_xl, xr_t=t_gs_xr,
                           src_d=D["gs_src"], dst_d=D["gs_dst"], ea_d=D["gs_ea"],
                           dr_d=D["gs_dr"], we_sb=S["we_gs"], att_sb=S["att_gs"],
                           slab=gs_slab, vd=cfg.n_s)
    nc.compile()
    return nc


def _build_launch2(cfg: Cfg, g_gs, w_gs):
    n_gs = w_gs * g_gs * 128
    hc1, hc3 = cfg.hc1, cfg.hc3
    nc = bacc.Bacc("TRN2", target_bir_lowering=False, debug=False,
                   num_devices=cfg.ncores)
    dI, dO = "ExternalInput", "ExternalOutput"
    D = {}
    def di(name, shape, dt):
        D[name] = nc.dram_tensor(name, list(shape), dt, kind=dI)
        return D[name]

    di("x1gT", (128, cfg.gsh), BF16)
    di("x1mT", (128, cfg.n_s), BF16)
    di("ident", (128, 128), BF16)
    di("iota", (128, 128), BF16)
    di("ones", (1, 128), BF16)
    di("w3_l", (hc1, hc3), BF16)
    di("w3_r", (hc1, hc3), BF16)
    di("b3_l", (1, hc3), BF16)
    di("b3_r", (1, hc3), BF16)
    di("we3", (1, hc3), BF16)
    di("att3", (128, 512), BF16)
    di("wsl3", (hc1, cfg.c3), BF16)
    di("gs_src", (128, n_gs // 16), I16)
    di("gs_dst", (128, n_gs // 16), I16)
    di("gs_dr", (128, n_gs // 128), BF16)
    di("gs_ea", (1, n_gs), BF16)

    l3_slab = nc.dram_tensor("l3_slab", [cfg.n_s, hc3 + 4], F32, kind=dO)
    sl3 = nc.dram_tensor("sl3", [cfg.c3, cfg.n_s], F32, kind=dO)
    t3_xl = nc.dram_tensor("t3_xl", [cfg.gsh, hc3], BF16)
    t3_xr = nc.dram_tensor("t3_xr", [cfg.n_s, hc3], BF16)

    with tile.TileContext(nc) as tc:
        import contextlib
        with contextlib.ExitStack() as ctx:
            persist = ctx.enter_context(tc.tile_pool(name="persist", bufs=1))
            sb_pool = ctx.enter_context(tc.tile_pool(name="sb", bufs=3))
            ps_pool = ctx.enter_context(
                tc.tile_pool(name="ps", bufs=2, space="PSUM"))
            win_pool = ctx.enter_context(
                tc.tile_pool(name="win", bufs=2, space="PSUM"))
            gat_pool = ctx.enter_context(tc.tile_pool(name="gat", bufs=2))

            nc.gpsimd.load_library(library_config.mlp)

            S = {}
            for name in ["x1gT", "x1mT", "ident", "iota", "ones", "w3_l",
                         "w3_r", "b3_l", "b3_r", "we3", "att3", "wsl3"]:
                d = D[name]
                S[name] = _load_sb(nc, persist, name, d.shape, d.dtype, d)

            tp = (sb_pool, ps_pool)
            _emit_table(nc, tp, S["x1gT"], S["w3_l"], S["b3_l"], S["ones"],
                        t3_xl, cfg.gsh, hc3)
            _emit_table(nc, tp, S["x1mT"], S["w3_r"], S["b3_r"], S["ones"],
                        t3_xr, cfg.n_s, hc3)
            _emit_sl(nc, tp, S["wsl3"], S["x1mT"], sl3, cfg.c3, cfg.n_s)

            pools = (sb_pool, ps_pool, gat_pool, win_pool)
            consts = (S["ident"], S["iota"])
            _emit_pass(nc, tc, pools, consts, n=n_gs, w_count=w_gs, g=g_gs,
                       fdim=hc3, h=cfg.h, xl_t=t3_xl, xr_t=t3_xr,
                       src_d=D["gs_src"], dst_d=D["gs_dst"], ea_d=D["gs_ea"],
                       dr_d=D["gs_dr"], we_sb=S["we3"], att_sb=S["att3"],
                       slab=l3_slab, vd=cfg.n_s)
    nc.compile()
    return nc


def _host_prep(inputs, cfg: Cfg):
    nco, gsh = cfg.ncores, cfg.gsh
    sg_src = np.asarray(inputs["sg_src"]).astype(np.int64)
    sg_dst = np.asarray(inputs["sg_dst"]).astype(np.int64)
    gs_src = np.asarray(inputs["gs_src"]).astype(np.int64)
    gs_dst = np.asarray(inputs["gs_dst"]).astype(np.int64)
    ea_sg = f32(inputs["ea_sg"]).ravel()
    ea_gs = f32(inputs["ea_gs"]).ravel()

    w_sg = math.ceil(gsh / WIN)
    w_gs = math.ceil(cfg.n_s / WIN)

    # per-core packing; first determine G (max window load) across cores
    per_core = []
    g_sg = g_gs = 1
    for c in range(nco):
        m_sg = (sg_dst // gsh) == c
        m_gs = (gs_src // gsh) == c
        e_sg = (sg_src[m_sg], sg_dst[m_sg] - c * gsh, ea_sg[m_sg])
        e_gs = (gs_src[m_gs] - c * gsh, gs_dst[m_gs], ea_gs[m_gs])
        per_core.append((e_sg, e_gs))
        cnt1 = np.bincount(e_sg[1] // WIN, minlength=w_sg)
        cnt2 = np.bincount(e_gs[1] // WIN, minlength=w_gs)
        g_sg = max(g_sg, math.ceil(cnt1.max() / 128))
        g_gs = max(g_gs, math.ceil(cnt2.max() / 128))
    n_sg = w_sg * g_sg * 128
    n_gs = w_gs * g_gs * 128

    core_edges = []
    for c in range(nco):
        (s1, d1, e1), (s2, d2, e2) = per_core[c]
        sg = _pack_edges(s1, d1, e1, w_sg, g_sg)
        gs = _pack_edges(s2, d2, e2, w_gs, g_gs)
        core_edges.append((sg, gs))

    hc1 = cfg.hc1
    common1 = {
        "xmT": bf(f32(inputs["x_mrna"]).T),
        "ident": bf(np.eye(128, dtype=np.float32)),
        "iota": bf(np.tile(np.arange(128, dtype=np.float32), (128, 1))),
        "ones": bf(np.ones((1, 128), np.float32)),
        "wsl1": bf(inputs["Wsl1"]),
    }
    for r in ("sg", "gs"):
        bo = f32(inputs[f"bo1_{r}"])
        common1[f"w_{r}_l"] = bf(inputs[f"Wl1_{r}"])
        common1[f"w_{r}_r"] = bf(inputs[f"Wr1_{r}"])
        common1[f"b_{r}_l"] = bf((f32(inputs[f"bl1_{r}"]) + bo).reshape(1, hc1))
        common1[f"b_{r}_r"] = bf((f32(inputs[f"br1_{r}"]) - bo).reshape(1, hc1))
        common1[f"we_{r}"] = bf(f32(inputs[f"We1_{r}"]).reshape(1, hc1))
        att = f32(inputs[f"att1_{r}"]).reshape(hc1)
        common1[f"att_{r}"] = bf(np.tile(np.tile(att, 512 // hc1), (128, 1)))

    xg = f32(inputs["x_gene"])
    in_maps1 = []
    for c in range(nco):
        sg, gs = core_edges[c]
        m = dict(common1)
        m["xgT"] = bf(xg[c * gsh : (c + 1) * gsh].T)
        m.update(_edge_inputs("sg", *sg, n_sg))
        m.update(_edge_inputs("gs", *gs, n_gs))
        in_maps1.append(m)

    return {
        "w_sg": w_sg, "w_gs": w_gs, "g_sg": g_sg, "g_gs": g_gs,
        "n_sg": n_sg, "n_gs": n_gs, "in_maps1": in_maps1,
        "core_edges": core_edges,
    }


def _host_mid(cfg: Cfg, res1, meta, inputs):
    h, c1, hc1 = cfg.h, cfg.c1, cfg.hc1

    def safediv(ws, dn, bo):
        dnr = np.repeat(dn, c1, axis=1)
        return np.where(dnr > 0, ws / np.where(dnr > 0, dnr, 1.0),
                        bo.reshape(1, hc1))

    xg_parts = []
    bo_sg = f32(inputs["bo1_sg"])
    for c in range(cfg.ncores):
        slab = res1.results[c]["sg_slab"]
        xg_parts.append(
            np.maximum(safediv(slab[:, :hc1], slab[:, hc1:], bo_sg), 0.0))
    x1_gene = np.concatenate(xg_parts, axis=0)

    ws = sum(res1.results[c]["gs_slab"][:, :hc1] for c in range(cfg.ncores))
    dn = sum(res1.results[c]["gs_slab"][:, hc1:] for c in range(cfg.ncores))
    sl1 = res1.results[0]["sl1"].T + f32(inputs["bsl1"])  # [n_s, c1]
    x1_mrna = np.maximum(
        safediv(ws, dn, f32(inputs["bo1_gs"])) + np.tile(sl1, (1, h)), 0.0)
    return x1_gene, x1_mrna


def _host_prep2(cfg: Cfg, inputs, meta, x1_gene, x1_mrna):
    hc3 = cfg.hc3
    bo3 = np.tile(f32(inputs["bo3_gs"]), cfg.h)
    common = {
        "x1mT": bf(x1_mrna.T),
        "ident": bf(np.eye(128, dtype=np.float32)),
        "iota": bf(np.tile(np.arange(128, dtype=np.float32), (128, 1))),
        "ones": bf(np.ones((1, 128), np.float32)),
        "w3_l": bf(inputs["Wl3_gs"]),
        "w3_r": bf(inputs["Wr3_gs"]),
        "b3_l": bf((f32(inputs["bl3_gs"]) + bo3).reshape(1, hc3)),
        "b3_r": bf((f32(inputs["br3_gs"]) - bo3).reshape(1, hc3)),
        "we3": bf(f32(inputs["We3_gs"]).reshape(1, hc3)),
        "att3": bf(np.tile(np.tile(f32(inputs["att3_gs"]).reshape(hc3),
                                    512 // hc3), (128, 1))),
        "wsl3": bf(inputs["Wsl3"]),
    }
    in_maps2 = []
    for c in range(cfg.ncores):
        _, gs = meta["core_edges"][c]
        m = dict(common)
        m["x1gT"] = bf(x1_gene[c * cfg.gsh : (c + 1) * cfg.gsh].T)
        m.update(_edge_inputs("gs", *gs, meta["n_gs"]))
        in_maps2.append(m)
    return in_maps2


def _host_final(cfg: Cfg, res2, inputs):
    hc3, c3 = cfg.hc3, cfg.c3
    ws = sum(res2.results[c]["l3_slab"][:, :hc3] for c in range(cfg.ncores))
    dn = sum(res2.results[c]["l3_slab"][:, hc3:] for c in range(cfg.ncores))
    sl3 = res2.results[0]["sl3"].T + f32(inputs["bsl3"])  # [n_s, c3]
    dn3 = dn[:, :, None]
    out = np.where(dn3 > 0,
                   ws.reshape(cfg.n_s, cfg.h, c3) / np.where(dn3 > 0, dn3, 1.0),
                   f32(inputs["bo3_gs"]).reshape(1, 1, c3))
    return np.maximum(out.mean(axis=1) + sl3, 0.0).astype(np.float32)


_trace = [False]


def kernel(**inputs):
    cfg = Cfg()
    meta = _host_prep(inputs, cfg)
    nc1 = _build_launch1(cfg, meta["g_sg"], meta["g_gs"], meta["w_sg"],
                         meta["w_gs"])
    cores = list(range(cfg.ncores))
    res1 = run_bass_kernel_spmd(nc1, meta["in_maps1"], cores,
                                trace=_trace[0])
    x1_gene, x1_mrna = _host_mid(cfg, res1, meta, inputs)
    in_maps2 = _host_prep2(cfg, inputs, meta, x1_gene, x1_mrna)
    nc2 = _build_launch2(cfg, meta["g_gs"], meta["w_gs"])
    res2 = run_bass_kernel_spmd(nc2, in_maps2, cores, trace=_trace[0])
    out = _host_final(cfg, res2, inputs)
    kernel.last = {"res1": res1, "res2": res2, "x1_gene": x1_gene,
                   "x1_mrna": x1_mrna, "meta": meta}
    return out


# revision 15
# speedup vs baseline: 1.2945x; 1.2945x over previous
"""BiRGAT (2-layer heterogeneous GATv2) on 8 Trainium2 NeuronCores.

Sharding:
  - sg relation (mrna->gene): edges dst-sharded by gene range (6250/core);
    segment softmax is core-local.
  - gs relation (gene->mrna) and layer-3: edges src-sharded by the same gene
    ranges; each core produces partial per-dst (mrna) weighted sums + softmax
    denominators which the host reduces (the unshard step).
Two launches: L1 (both relations) -> host combines x1 -> L2 (layer 3).

Device pipeline per 128-edge group (edges on partitions, features on free):
  dma_gather xl[src], xr[dst] rows (bf16 tables in DRAM)
  PSUM s = ea (x) We + I@xl + I@xr   (3 matmuls)
  m = LeakyRelu(s) [ACT], logits = reduce(m*att) [DVE], ex = Exp(logits) [ACT]
  aug = [xl * ex_bcast | ex]  (DVE)
  window matmul: psum_w += sel^T @ aug, sel = is_equal(dst_rel, iota)
  (windows = 127 consecutive dst ids + 1 pad slot; edge list padded per
   window to G*128 slots so the whole schedule is static)
Softmax uses exp without max subtraction (logits are provably tiny here);
output bias bo is folded into the xl table (+bo) and xr table (-bo) so the
alpha-weighted sum directly yields out + bo (softmax weights sum to 1).
"""

import math
from dataclasses import dataclass, field

import ml_dtypes
import numpy as np

import concourse.bacc as bacc
import concourse.bass as bass
import concourse.mybir as mybir
import concourse.tile as tile
from concourse import library_config
from concourse.bass_utils import run_bass_kernel_spmd

F32 = mybir.dt.float32
BF16 = mybir.dt.bfloat16
I16 = mybir.dt.int16
NEG_SLOPE = 0.2
WIN = 127  # real dst ids per window (slot 127 = pad)
CH = 4096  # edges per gather chunk

bf = lambda a: np.ascontiguousarray(a).astype(ml_dtypes.bfloat16)
f32 = lambda a: np.ascontiguousarray(a).astype(np.float32)


@dataclass
class Cfg:
    n_s: int = 4000
    n_g: int = 50000
    e: int = 250000
    d: int = 128
    h: int = 4
    c1: int = 32
    c3: int = 64
    ncores: int = 8

    @property
    def hc1(self):
        return self.h * self.c1

    @property
    def hc3(self):
        return self.h * self.c3

    @property
    def gsh(self):
        return self.n_g // self.ncores


def _wrap_idx(idx):
    """[n] -> [128, n//16] int16, wrapped in 16 partitions, replicated x8."""
    n = len(idx)
    w = idx.reshape(n // 16, 16).T.astype(np.int16)
    return np.ascontiguousarray(np.tile(w, (8, 1)))


def _pm(v, n):
    """[n] -> [128, n//128] partition-major gather layout."""
    return np.ascontiguousarray(v.reshape(n // 128, 128).T)


def _pack_edges(srcl, dstl, ea, w_count, g):
    """Sort by dst, lay out into per-window slabs of g*128 slots."""
    n = w_count * g * 128
    order = np.argsort(dstl, kind="stable")
    s, d, e = srcl[order], dstl[order], ea[order]
    w = d // WIN
    cnt = np.bincount(w, minlength=w_count)
    assert cnt.max() <= g * 128, (cnt.max(), g * 128)
    start = np.zeros(w_count, np.int64)
    start[1:] = np.cumsum(cnt)[:-1]
    slot = w * g * 128 + (np.arange(len(d)) - start[w])
    S = np.zeros(n, np.int64)
    D = np.zeros(n, np.int64)
    Dr = np.full(n, WIN, np.int64)
    Ea = np.zeros(n, np.float32)
    S[slot] = s
    D[slot] = d
    Dr[slot] = d - WIN * w
    Ea[slot] = e
    return S, D, Dr, Ea


def _edge_inputs(prefix, S, D, Dr, Ea, n):
    return {
        f"{prefix}_src": _wrap_idx(S),
        f"{prefix}_dst": _wrap_idx(D),
        f"{prefix}_dr": bf(_pm(Dr, n)),
        f"{prefix}_ea": bf(Ea.reshape(1, n)),
    }


def _emit_table(nc, pools, xT_sb, w_sb, b_sb, ones_sb, table, v, fdim):
    """table[v, fdim] (bf16 DRAM) = xT.T @ w + b, chunked by 128 rows."""
    sb_pool, ps_pool = pools
    for i in range(math.ceil(v / 128)):
        m = min(128, v - i * 128)
        ps = ps_pool.tile([128, fdim], F32, tag="tfps")
        nc.tensor.matmul(
            out=ps[:m], lhsT=xT_sb[:, i * 128 : i * 128 + m], rhs=w_sb[:],
            start=True, stop=False,
        )
        nc.tensor.matmul(
            out=ps[:m], lhsT=ones_sb[0:1, :m], rhs=b_sb[:], start=False, stop=True,
        )
        sb = sb_pool.tile([128, fdim], BF16, tag="tfsb")
        nc.scalar.copy(sb[:m], ps[:m])
        nc.sync.dma_start(out=table[i * 128 : i * 128 + m, :], in_=sb[:m])


def _emit_pass(nc, tc, pools, consts, *, n, w_count, g, fdim, h, xl_t, xr_t,
               src_d, dst_d, ea_d, dr_d, we_sb, att_sb, slab, vd, level=3):
    """One GAT edge pass."""
    sb_pool, ps_pool, gat_pool, win_pool = pools
    ident_sb, iota_sb = consts
    c = fdim // h
    bgrp = 512 // fdim  # groups per s-psum bank
    fa = fdim + 4  # aug width (features + per-head ex)

    psw = None
    for c0 in range(0, n, CH):
        csz = min(CH, n - c0)
        ngr = csz // 128
        xi = gat_pool.tile([128, csz // 16], I16, tag="xi")
        ri = gat_pool.tile([128, csz // 16], I16, tag="ri")
        nc.sync.dma_start(out=xi[:], in_=src_d[:, c0 // 16 : (c0 + csz) // 16])
        nc.sync.dma_start(out=ri[:], in_=dst_d[:, c0 // 16 : (c0 + csz) // 16])
        xlg = gat_pool.tile([128, ngr, fdim], BF16, tag="xlg")
        xrg = gat_pool.tile([128, ngr, fdim], BF16, tag="xrg")
        GC = 1024
        for q0 in range(0, csz, GC):
            qs = min(GC, csz - q0)
            qg = q0 // 128
            nc.gpsimd.dma_gather(
                xlg[:, qg : qg + qs // 128, :], xl_t[:],
                xi[:, q0 // 16 : (q0 + qs) // 16], qs, qs, fdim)
            nc.gpsimd.dma_gather(
                xrg[:, qg : qg + qs // 128, :], xr_t[:],
                ri[:, q0 // 16 : (q0 + qs) // 16], qs, qs, fdim)
        ea_sb = gat_pool.tile([1, csz], BF16, tag="ea")
        dr_sb = gat_pool.tile([128, ngr], BF16, tag="dr")
        nc.sync.dma_start(out=ea_sb[:], in_=ea_d[:, c0 : c0 + csz])
        nc.sync.dma_start(out=dr_sb[:], in_=dr_d[:, c0 // 128 : c0 // 128 + ngr])

        if level == 0:
            if c0 == 0:
                dbg = gat_pool.tile([128, 512], F32, tag="dbg")
                nc.vector.tensor_copy(out=dbg[:, :fdim], in_=xlg[:, 0, :])
                nc.vector.tensor_copy(out=dbg[:, fdim : fdim + 4], in_=xrg[:, 0, 0:4])
                nc.sync.dma_start(out=slab[0:128, :], in_=dbg[:128, :fa])
            continue
        for b0 in range(0, ngr, bgrp):
            nb = min(bgrp, ngr - b0)
            sps = ps_pool.tile([128, 512], F32, tag="sps")
            for j in range(b0, b0 + nb):
                col = (j - b0) * fdim
                o = sps[:, col : col + fdim]
                nc.tensor.matmul(
                    out=o, lhsT=ea_sb[0:1, j * 128 : (j + 1) * 128],
                    rhs=we_sb[:], start=True, stop=False,
                )
                nc.tensor.matmul(
                    out=o, lhsT=ident_sb[:], rhs=xlg[:, j, :], start=False,
                    stop=False,
                )
                nc.tensor.matmul(
                    out=o, lhsT=ident_sb[:], rhs=xrg[:, j, :], start=False,
                    stop=True,
                )
            nf = nb * fdim
            r_sb = gat_pool.tile([128, 512], BF16, tag="r")
            nc.scalar.mul(r_sb[:, :nf], sps[:, :nf], NEG_SLOPE)
            m_sb = gat_pool.tile([128, 512], BF16, tag="m")
            nc.vector.tensor_tensor(
                out=m_sb[:, :nf], in0=sps[:, :nf], in1=r_sb[:, :nf],
                op=mybir.AluOpType.max,
            )
            am = gat_pool.tile([128, 512], BF16, tag="am")
            nc.vector.tensor_tensor(
                out=am[:, :nf], in0=m_sb[:, :nf],
                in1=att_sb[:, :nf],
                op=mybir.AluOpType.mult,
            )
            lgt = gat_pool.tile([128, bgrp * h], F32, tag="lgt")
            nc.vector.tensor_reduce(
                out=lgt[:, : nb * h],
                in_=am[:, :nf].rearrange("p (x c) -> p x c", c=c),
                axis=mybir.AxisListType.X, op=mybir.AluOpType.add,
            )
            if level == 1:
                dbg = gat_pool.tile([128, 512], F32, tag="dbg")
                nc.scalar.copy(dbg[:, :nf], m_sb[:, :nf])
                if b0 == 0 and c0 == 0:
                    nc.sync.dma_start(out=slab[0:128, :], in_=dbg[:128, :fa])
                continue
            ex = gat_pool.tile([128, bgrp * h], BF16, tag="ex")
            nc.scalar.activation(
                ex[:, : nb * h], lgt[:, : nb * h],
                mybir.ActivationFunctionType.Exp,
            )
            aug = gat_pool.tile([128, bgrp * fdim], BF16, tag="aug")
            nc.vector.tensor_tensor(
                out=aug[:, : nb * fdim].rearrange(
                    "p (b hh cc) -> p b hh cc", hh=h, cc=c),
                in0=xlg[:, b0 : b0 + nb, :].rearrange(
                    "p b (hh cc) -> p b hh cc", cc=c
                ),
                in1=ex[:, : nb * h].rearrange("p (b hh) -> p b hh", hh=h)
                .unsqueeze(3).to_broadcast([128, nb, h, c]),
                op=mybir.AluOpType.mult,
            )
            if level == 2:
                dbg = gat_pool.tile([128, 512], F32, tag="dbg")
                nc.scalar.copy(dbg[:, : nb * fdim], aug[:, : nb * fdim])
                if b0 == 0 and c0 == 0:
                    nc.sync.dma_start(out=slab[0:128, :], in_=dbg[:128, :fa])
                continue
            for j in range(b0, b0 + nb):
                grp = c0 // 128 + j
                wi, jw = divmod(grp, g)
                if jw == 0:
                    psw = win_pool.tile([128, fa], F32, tag="psw")
                sel = gat_pool.tile([128, 128], BF16, tag="sel")
                nc.vector.tensor_tensor(
                    out=sel[:],
                    in0=dr_sb[:, j : j + 1].to_broadcast([128, 128]),
                    in1=iota_sb[:],
                    op=mybir.AluOpType.is_equal,
                )
                nc.tensor.matmul(
                    out=psw[:, :fdim],
                    lhsT=sel[:],
                    rhs=aug[:, (j - b0) * fdim : (j - b0 + 1) * fdim],
                    start=(jw == 0), stop=False,
                    skip_group_check=True,
                )
                nc.tensor.matmul(
                    out=psw[:, fdim:fa],
                    lhsT=sel[:],
                    rhs=ex[:, (j - b0) * h : (j - b0 + 1) * h],
                    start=False, stop=(jw == g - 1),
                    skip_group_check=True,
                )
                if jw == g - 1:
                    rows = min(WIN, vd - WIN * wi)
                    osb = gat_pool.tile([128, fa], F32, tag="osb")
                    nc.scalar.copy(osb[:], psw[:])
                    nc.sync.dma_start(
                        out=slab[WIN * wi : WIN * wi + rows, :], in_=osb[:rows],
                    )


def _emit_sl(nc, pools, wsl_sb, xT_sb, out_d, cx, v):
    """out_d[cx, v] f32 = wsl.T @ x  (self-loop linear), chunked by 512."""
    sb_pool, ps_pool = pools
    for i in range(math.ceil(v / 512)):
        m = min(512, v - i * 512)
        ps = ps_pool.tile([cx, 512], F32, tag="slps")
        nc.tensor.matmul(
            out=ps[:, :m], lhsT=wsl_sb[:], rhs=xT_sb[:, i * 512 : i * 512 + m],
            start=True, stop=True,
        )
        sb = sb_pool.tile([cx, 512], F32, tag="slsb")
        nc.scalar.copy(sb[:, :m], ps[:, :m])
        nc.sync.dma_start(out=out_d[:, i * 512 : i * 512 + m], in_=sb[:, :m])


def _load_sb(nc, pool, name, arr_shape, dtype, dram):
    t = pool.tile(list(arr_shape), dtype, tag=name)
    nc.sync.dma_start(out=t[:], in_=dram[:])
    return t


def _build_launch1(cfg: Cfg, g_sg, g_gs, w_sg, w_gs, parts=("tables", "sl", "sg", "gs")):
    n_sg = w_sg * g_sg * 128
    n_gs = w_gs * g_gs * 128
    hc1 = cfg.hc1
    nc = bacc.Bacc("TRN2", target_bir_lowering=False, debug=False,
                   num_devices=cfg.ncores)
    dI, dO = "ExternalInput", "ExternalOutput"
    D = {}
    def di(name, shape, dt):
        D[name] = nc.dram_tensor(name, list(shape), dt, kind=dI)
        return D[name]

    di("xmT", (128, cfg.n_s), BF16)
    di("xgT", (128, cfg.gsh), BF16)
    di("ident", (128, 128), BF16)
    di("iota", (128, 128), BF16)
    di("ones", (1, 128), BF16)
    for r in ("sg", "gs"):
        di(f"w_{r}_l", (cfg.d, hc1), BF16)
        di(f"w_{r}_r", (cfg.d, hc1), BF16)
        di(f"b_{r}_l", (1, hc1), BF16)
        di(f"b_{r}_r", (1, hc1), BF16)
        di(f"we_{r}", (1, hc1), BF16)
        di(f"att_{r}", (128, 512), BF16)
    di("wsl1", (cfg.d, cfg.c1), BF16)
    for r, n in (("sg", n_sg), ("gs", n_gs)):
        di(f"{r}_src", (128, n // 16), I16)
        di(f"{r}_dst", (128, n // 16), I16)
        di(f"{r}_dr", (128, n // 128), BF16)
        di(f"{r}_ea", (1, n), BF16)

    sg_slab = nc.dram_tensor("sg_slab", [cfg.gsh, hc1 + 4], F32, kind=dO)
    gs_slab = nc.dram_tensor("gs_slab", [cfg.n_s, hc1 + 4], F32, kind=dO)
    sl1 = nc.dram_tensor("sl1", [cfg.c1, cfg.n_s], F32, kind=dO)

    t_sg_xl = nc.dram_tensor("t_sg_xl", [cfg.n_s, hc1], BF16)
    t_sg_xr = nc.dram_tensor("t_sg_xr", [cfg.gsh, hc1], BF16)
    t_gs_xl = nc.dram_tensor("t_gs_xl", [cfg.gsh, hc1], BF16)
    t_gs_xr = nc.dram_tensor("t_gs_xr", [cfg.n_s, hc1], BF16)

    with tile.TileContext(nc) as tc:
        import contextlib
        with contextlib.ExitStack() as ctx:
            persist = ctx.enter_context(tc.tile_pool(name="persist", bufs=1))
            sb_pool = ctx.enter_context(tc.tile_pool(name="sb", bufs=3))
            ps_pool = ctx.enter_context(
                tc.tile_pool(name="ps", bufs=2, space="PSUM"))
            win_pool = ctx.enter_context(
                tc.tile_pool(name="win", bufs=2, space="PSUM"))
            gat_pool = ctx.enter_context(tc.tile_pool(name="gat", bufs=2))

            nc.gpsimd.load_library(library_config.mlp)

            S = {}
            for name in ["xmT", "xgT", "ident", "iota", "ones", "wsl1",
                         "w_sg_l", "w_sg_r", "b_sg_l", "b_sg_r", "we_sg",
                         "att_sg", "w_gs_l", "w_gs_r", "b_gs_l", "b_gs_r",
                         "we_gs", "att_gs"]:
                d = D[name]
                S[name] = _load_sb(nc, persist, name, d.shape, d.dtype, d)

            tp = (sb_pool, ps_pool)
            if "tables" in parts:
                _emit_table(nc, tp, S["xmT"], S["w_sg_l"], S["b_sg_l"], S["ones"],
                            t_sg_xl, cfg.n_s, hc1)
                _emit_table(nc, tp, S["xgT"], S["w_sg_r"], S["b_sg_r"], S["ones"],
                            t_sg_xr, cfg.gsh, hc1)
                _emit_table(nc, tp, S["xgT"], S["w_gs_l"], S["b_gs_l"], S["ones"],
                            t_gs_xl, cfg.gsh, hc1)
                _emit_table(nc, tp, S["xmT"], S["w_gs_r"], S["b_gs_r"], S["ones"],
                            t_gs_xr, cfg.n_s, hc1)
            if "sl" in parts:
                _emit_sl(nc, tp, S["wsl1"], S["xmT"], sl1, cfg.c1, cfg.n_s)

            pools = (sb_pool, ps_pool, gat_pool, win_pool)
            consts = (S["ident"], S["iota"])
            if "sg" in parts:
                _emit_pass(nc, tc, pools, consts, n=n_sg, w_count=w_sg, g=g_sg,
                           fdim=hc1, h=cfg.h, xl_t=t_sg_xl, xr_t=t_sg_xr,
                           src_d=D["sg_src"], dst_d=D["sg_dst"], ea_d=D["sg_ea"],
                           dr_d=D["sg_dr"], we_sb=S["we_sg"], att_sb=S["att_sg"],
                           slab=sg_slab, vd=cfg.gsh)
            if "gs" in parts:
                _emit_pass(nc, tc, pools, consts, n=n_gs, w_count=w_gs, g=g_gs,
                           fdim=hc1, h=cfg.h, xl_t=t_gs_xl, xr_t=t_gs_xr,
                           src_d=D["gs_src"], dst_d=D["gs_dst"], ea_d=D["gs_ea"],
                           dr_d=D["gs_dr"], we_sb=S["we_gs"], att_sb=S["att_gs"],
                           slab=gs_slab, vd=cfg.n_s)
    nc.compile()
    return nc


def _build_launch2(cfg: Cfg, g_gs, w_gs):
    n_gs = w_gs * g_gs * 128
    hc1, hc3 = cfg.hc1, cfg.hc3
    nc = bacc.Bacc("TRN2", target_bir_lowering=False, debug=False,
                   num_devices=cfg.ncores)
    dI, dO = "ExternalInput", "ExternalOutput"
    D = {}
    def di(name, shape, dt):
        D[name] = nc.dram_tensor(name, list(shape), dt, kind=dI)
        return D[name]

    di("x1gT", (128, cfg.gsh), BF16)
    di("x1mT", (128, cfg.n_s), BF16)
    di("ident", (128, 128), BF16)
    di("iota", (128, 128), BF16)
    di("ones", (1, 128), BF16)
    di("w3_l", (hc1, hc3), BF16)
    di("w3_r", (hc1, hc3), BF16)
    di("b3_l", (1, hc3), BF16)
    di("b3_r", (1, hc3), BF16)
    di("we3", (1, hc3), BF16)
    di("att3", (128, 512), BF16)
    di("wsl3", (hc1, cfg.c3), BF16)
    di("gs_src", (128, n_gs // 16), I16)
    di("gs_dst", (128, n_gs // 16), I16)
    di("gs_dr", (128, n_gs // 128), BF16)
    di("gs_ea", (1, n_gs), BF16)

    l3_slab = nc.dram_tensor("l3_slab", [cfg.n_s, hc3 + 4], F32, kind=dO)
    sl3 = nc.dram_tensor("sl3", [cfg.c3, cfg.n_s], F32, kind=dO)
    t3_xl = nc.dram_tensor("t3_xl", [cfg.gsh, hc3], BF16)
    t3_xr = nc.dram_tensor("t3_xr", [cfg.n_s, hc3], BF16)

    with tile.TileContext(nc) as tc:
        import contextlib
        with contextlib.ExitStack() as ctx:
            persist = ctx.enter_context(tc.tile_pool(name="persist", bufs=1))
            sb_pool = ctx.enter_context(tc.tile_pool(name="sb", bufs=3))
            ps_pool = ctx.enter_context(
                tc.tile_pool(name="ps", bufs=2, space="PSUM"))
            win_pool = ctx.enter_context(
                tc.tile_pool(name="win", bufs=2, space="PSUM"))
            gat_pool = ctx.enter_context(tc.tile_pool(name="gat", bufs=2))

            nc.gpsimd.load_library(library_config.mlp)

            S = {}
            for name in ["x1gT", "x1mT", "ident", "iota", "ones", "w3_l",
                         "w3_r", "b3_l", "b3_r", "we3", "att3", "wsl3"]:
                d = D[name]
                S[name] = _load_sb(nc, persist, name, d.shape, d.dtype, d)

            tp = (sb_pool, ps_pool)
            _emit_table(nc, tp, S["x1gT"], S["w3_l"], S["b3_l"], S["ones"],
                        t3_xl, cfg.gsh, hc3)
            _emit_table(nc, tp, S["x1mT"], S["w3_r"], S["b3_r"], S["ones"],
                        t3_xr, cfg.n_s, hc3)
            _emit_sl(nc, tp, S["wsl3"], S["x1mT"], sl3, cfg.c3, cfg.n_s)

            pools = (sb_pool, ps_pool, gat_pool, win_pool)
            consts = (S["ident"], S["iota"])
            _emit_pass(nc, tc, pools, consts, n=n_gs, w_count=w_gs, g=g_gs,
                       fdim=hc3, h=cfg.h, xl_t=t3_xl, xr_t=t3_xr,
                       src_d=D["gs_src"], dst_d=D["gs_dst"], ea_d=D["gs_ea"],
                       dr_d=D["gs_dr"], we_sb=S["we3"], att_sb=S["att3"],
                       slab=l3_slab, vd=cfg.n_s)
    nc.compile()
    return nc


def _host_prep(inputs, cfg: Cfg):
    nco, gsh = cfg.ncores, cfg.gsh
    sg_src = np.asarray(inputs["sg_src"]).astype(np.int64)
    sg_dst = np.asarray(inputs["sg_dst"]).astype(np.int64)
    gs_src = np.asarray(inputs["gs_src"]).astype(np.int64)
    gs_dst = np.asarray(inputs["gs_dst"]).astype(np.int64)
    ea_sg = f32(inputs["ea_sg"]).ravel()
    ea_gs = f32(inputs["ea_gs"]).ravel()

    w_sg = math.ceil(gsh / WIN)
    w_gs = math.ceil(cfg.n_s / WIN)

    # per-core packing; first determine G (max window load) across cores
    per_core = []
    g_sg = g_gs = 1
    for c in range(nco):
        m_sg = (sg_dst // gsh) == c
        m_gs = (gs_src // gsh) == c
        e_sg = (sg_src[m_sg], sg_dst[m_sg] - c * gsh, ea_sg[m_sg])
        e_gs = (gs_src[m_gs] - c * gsh, gs_dst[m_gs], ea_gs[m_gs])
        per_core.append((e_sg, e_gs))
        cnt1 = np.bincount(e_sg[1] // WIN, minlength=w_sg)
        cnt2 = np.bincount(e_gs[1] // WIN, minlength=w_gs)
        g_sg = max(g_sg, math.ceil(cnt1.max() / 128))
        g_gs = max(g_gs, math.ceil(cnt2.max() / 128))
    n_sg = w_sg * g_sg * 128
    n_gs = w_gs * g_gs * 128

    core_edges = []
    for c in range(nco):
        (s1, d1, e1), (s2, d2, e2) = per_core[c]
        sg = _pack_edges(s1, d1, e1, w_sg, g_sg)
        gs = _pack_edges(s2, d2, e2, w_gs, g_gs)
        core_edges.append((sg, gs))

    hc1 = cfg.hc1
    common1 = {
        "xmT": bf(f32(inputs["x_mrna"]).T),
        "ident": bf(np.eye(128, dtype=np.float32)),
        "iota": bf(np.tile(np.arange(128, dtype=np.float32), (128, 1))),
        "ones": bf(np.ones((1, 128), np.float32)),
        "wsl1": bf(inputs["Wsl1"]),
    }
    for r in ("sg", "gs"):
        bo = f32(inputs[f"bo1_{r}"])
        common1[f"w_{r}_l"] = bf(inputs[f"Wl1_{r}"])
        common1[f"w_{r}_r"] = bf(inputs[f"Wr1_{r}"])
        common1[f"b_{r}_l"] = bf((f32(inputs[f"bl1_{r}"]) + bo).reshape(1, hc1))
        common1[f"b_{r}_r"] = bf((f32(inputs[f"br1_{r}"]) - bo).reshape(1, hc1))
        common1[f"we_{r}"] = bf(f32(inputs[f"We1_{r}"]).reshape(1, hc1))
        att = f32(inputs[f"att1_{r}"]).reshape(hc1)
        common1[f"att_{r}"] = bf(np.tile(np.tile(att, 512 // hc1), (128, 1)))

    xg = f32(inputs["x_gene"])
    in_maps1 = []
    for c in range(nco):
        sg, gs = core_edges[c]
        m = dict(common1)
        m["xgT"] = bf(xg[c * gsh : (c + 1) * gsh].T)
        m.update(_edge_inputs("sg", *sg, n_sg))
        m.update(_edge_inputs("gs", *gs, n_gs))
        in_maps1.append(m)

    return {
        "w_sg": w_sg, "w_gs": w_gs, "g_sg": g_sg, "g_gs": g_gs,
        "n_sg": n_sg, "n_gs": n_gs, "in_maps1": in_maps1,
        "core_edges": core_edges,
    }


def _host_mid(cfg: Cfg, res1, meta, inputs):
    h, c1, hc1 = cfg.h, cfg.c1, cfg.hc1

    def safediv(ws, dn, bo):
        dnr = np.repeat(dn, c1, axis=1)
        return np.where(dnr > 0, ws / np.where(dnr > 0, dnr, 1.0),
                        bo.reshape(1, hc1))

    xg_parts = []
    bo_sg = f32(inputs["bo1_sg"])
    for c in range(cfg.ncores):
        slab = res1.results[c]["sg_slab"]
        xg_parts.append(
            np.maximum(safediv(slab[:, :hc1], slab[:, hc1:], bo_sg), 0.0))
    x1_gene = np.concatenate(xg_parts, axis=0)

    ws = sum(res1.results[c]["gs_slab"][:, :hc1] for c in range(cfg.ncores))
    dn = sum(res1.results[c]["gs_slab"][:, hc1:] for c in range(cfg.ncores))
    sl1 = res1.results[0]["sl1"].T + f32(inputs["bsl1"])  # [n_s, c1]
    x1_mrna = np.maximum(
        safediv(ws, dn, f32(inputs["bo1_gs"])) + np.tile(sl1, (1, h)), 0.0)
    return x1_gene, x1_mrna


def _host_prep2(cfg: Cfg, inputs, meta, x1_gene, x1_mrna):
    hc3 = cfg.hc3
    bo3 = np.tile(f32(inputs["bo3_gs"]), cfg.h)
    common = {
        "x1mT": bf(x1_mrna.T),
        "ident": bf(np.eye(128, dtype=np.float32)),
        "iota": bf(np.tile(np.arange(128, dtype=np.float32), (128, 1))),
        "ones": bf(np.ones((1, 128), np.float32)),
        "w3_l": bf(inputs["Wl3_gs"]),
        "w3_r": bf(inputs["Wr3_gs"]),
        "b3_l": bf((f32(inputs["bl3_gs"]) + bo3).reshape(1, hc3)),
        "b3_r": bf((f32(inputs["br3_gs"]) - bo3).reshape(1, hc3)),
        "we3": bf(f32(inputs["We3_gs"]).reshape(1, hc3)),
        "att3": bf(np.tile(np.tile(f32(inputs["att3_gs"]).reshape(hc3),
                                    512 // hc3), (128, 1))),
        "wsl3": bf(inputs["Wsl3"]),
    }
    in_maps2 = []
    for c in range(cfg.ncores):
        _, gs = meta["core_edges"][c]
        m = dict(common)
        m["x1gT"] = bf(x1_gene[c * cfg.gsh : (c + 1) * cfg.gsh].T)
        m.update(_edge_inputs("gs", *gs, meta["n_gs"]))
        in_maps2.append(m)
    return in_maps2


def _host_final(cfg: Cfg, res2, inputs):
    hc3, c3 = cfg.hc3, cfg.c3
    ws = sum(res2.results[c]["l3_slab"][:, :hc3] for c in range(cfg.ncores))
    dn = sum(res2.results[c]["l3_slab"][:, hc3:] for c in range(cfg.ncores))
    sl3 = res2.results[0]["sl3"].T + f32(inputs["bsl3"])  # [n_s, c3]
    dn3 = dn[:, :, None]
    out = np.where(dn3 > 0,
                   ws.reshape(cfg.n_s, cfg.h, c3) / np.where(dn3 > 0, dn3, 1.0),
                   f32(inputs["bo3_gs"]).reshape(1, 1, c3))
    return np.maximum(out.mean(axis=1) + sl3, 0.0).astype(np.float32)


_trace = [False]


def kernel(**inputs):
    cfg = Cfg()
    meta = _host_prep(inputs, cfg)
    nc1 = _build_launch1(cfg, meta["g_sg"], meta["g_gs"], meta["w_sg"],
                         meta["w_gs"])
    cores = list(range(cfg.ncores))
    res1 = run_bass_kernel_spmd(nc1, meta["in_maps1"], cores,
                                trace=_trace[0])
    x1_gene, x1_mrna = _host_mid(cfg, res1, meta, inputs)
    in_maps2 = _host_prep2(cfg, inputs, meta, x1_gene, x1_mrna)
    nc2 = _build_launch2(cfg, meta["g_gs"], meta["w_gs"])
    res2 = run_bass_kernel_spmd(nc2, in_maps2, cores, trace=_trace[0])
    out = _host_final(cfg, res2, inputs)
    kernel.last = {"res1": res1, "res2": res2, "x1_gene": x1_gene,
                   "x1_mrna": x1_mrna, "meta": meta}
    return out


# revision 18
# speedup vs baseline: 3.0691x; 2.3709x over previous
"""BiRGAT (2-layer heterogeneous GATv2) on 8 Trainium2 NeuronCores.

Sharding:
  - sg relation (mrna->gene): edges dst-sharded by gene range (6250/core);
    segment softmax is core-local.
  - gs relation (gene->mrna) and layer-3: edges src-sharded by the same gene
    ranges; each core produces partial per-dst (mrna) weighted sums + softmax
    denominators which the host reduces (the unshard step).
Two launches: L1 (both relations) -> host combines x1 -> L2 (layer 3).

Device pipeline per 128-edge group (edges on partitions, features on free):
  dma_gather xl[src], xr[dst] rows (bf16 tables in DRAM)
  PSUM s = ea (x) We + I@xl + I@xr   (3 matmuls)
  m = LeakyRelu(s) [ACT], logits = reduce(m*att) [DVE], ex = Exp(logits) [ACT]
  aug = [xl * ex_bcast | ex]  (DVE)
  window matmul: psum_w += sel^T @ aug, sel = is_equal(dst_rel, iota)
  (windows = 127 consecutive dst ids + 1 pad slot; edge list padded per
   window to G*128 slots so the whole schedule is static)
Softmax uses exp without max subtraction (logits are provably tiny here);
output bias bo is folded into the xl table (+bo) and xr table (-bo) so the
alpha-weighted sum directly yields out + bo (softmax weights sum to 1).
"""

import math
from dataclasses import dataclass, field

import ml_dtypes
import numpy as np

import concourse.bacc as bacc
import concourse.bass as bass
import concourse.mybir as mybir
import concourse.tile as tile
from concourse import library_config
from concourse.bass_utils import run_bass_kernel_spmd

F32 = mybir.dt.float32
BF16 = mybir.dt.bfloat16
I16 = mybir.dt.int16
NEG_SLOPE = 0.2
WIN = 127  # real dst ids per window (slot 127 = pad)
CH = 4096  # edges per gather chunk

bf = lambda a: np.ascontiguousarray(a).astype(ml_dtypes.bfloat16)
f32 = lambda a: np.ascontiguousarray(a).astype(np.float32)


@dataclass
class Cfg:
    n_s: int = 4000
    n_g: int = 50000
    e: int = 250000
    d: int = 128
    h: int = 4
    c1: int = 32
    c3: int = 64
    ncores: int = 8

    @property
    def hc1(self):
        return self.h * self.c1

    @property
    def hc3(self):
        return self.h * self.c3

    @property
    def gsh(self):
        return self.n_g // self.ncores


def _wrap_idx(idx):
    """[n] -> [128, n//16] int16, wrapped in 16 partitions, replicated x8."""
    n = len(idx)
    w = idx.reshape(n // 16, 16).T.astype(np.int16)
    return np.ascontiguousarray(np.tile(w, (8, 1)))


def _pm(v, n):
    """[n] -> [128, n//128] partition-major gather layout."""
    return np.ascontiguousarray(v.reshape(n // 128, 128).T)


def _pack_edges(srcl, dstl, ea, w_count, g):
    """Sort by dst, lay out into per-window slabs of g*128 slots."""
    n = w_count * g * 128
    order = np.argsort(dstl, kind="stable")
    s, d, e = srcl[order], dstl[order], ea[order]
    w = d // WIN
    cnt = np.bincount(w, minlength=w_count)
    assert cnt.max() <= g * 128, (cnt.max(), g * 128)
    start = np.zeros(w_count, np.int64)
    start[1:] = np.cumsum(cnt)[:-1]
    slot = w * g * 128 + (np.arange(len(d)) - start[w])
    S = np.zeros(n, np.int64)
    D = np.zeros(n, np.int64)
    Dr = np.full(n, WIN, np.int64)
    Ea = np.zeros(n, np.float32)
    S[slot] = s
    D[slot] = d
    Dr[slot] = d - WIN * w
    Ea[slot] = e
    return S, D, Dr, Ea


def _edge_inputs(prefix, S, D, Dr, Ea, n):
    return {
        f"{prefix}_src": _wrap_idx(S),
        f"{prefix}_dst": _wrap_idx(D),
        f"{prefix}_dr": bf(_pm(Dr, n)),
        f"{prefix}_ea": bf(Ea.reshape(1, n)),
    }


def _emit_table(nc, pools, xT_sb, w_sb, b_sb, ones_sb, table, v, fdim):
    """table[v, fdim] (bf16 DRAM) = xT.T @ w + b, chunked by 128 rows."""
    sb_pool, ps_pool = pools
    for i in range(math.ceil(v / 128)):
        m = min(128, v - i * 128)
        ps = ps_pool.tile([128, fdim], F32, tag="tfps")
        nc.tensor.matmul(
            out=ps[:m], lhsT=xT_sb[:, i * 128 : i * 128 + m], rhs=w_sb[:],
            start=True, stop=False,
        )
        nc.tensor.matmul(
            out=ps[:m], lhsT=ones_sb[0:1, :m], rhs=b_sb[:], start=False, stop=True,
        )
        sb = sb_pool.tile([128, fdim], BF16, tag="tfsb")
        nc.scalar.copy(sb[:m], ps[:m])
        nc.sync.dma_start(out=table[i * 128 : i * 128 + m, :], in_=sb[:m])


def _emit_pass(nc, tc, pools, consts, *, n, w_count, g, fdim, h,
               xlT_d, xrT_d, eo_d, dr_d, wl_sb, wr_sb, eo_s_sb, slab, vd,
               level=3):
    """One GAT edge pass (v2): raw src/dst features host-gathered per edge.

    Per 128-edge group j (edges on partitions after PE transform):
      sps  = xlT_j.T @ Wl + xrT_j.T @ Wr + eo_j.T @ [We; bl+br]   (PSUM)
      pxl  = xlT_j.T @ Wl                                          (PSUM)
      m = lrelu(sps); logits = reduce(m*att); ex = exp(logits)
      aug = pxl * exb ; psw[:, :F] += sel^T @ aug ; psw[:, F:] += sel^T @ ex
    """
    sb_pool, ps_pool, gat_pool, win_pool = pools
    iota_sb, att_sb = consts
    c = fdim // h
    bgrp = 512 // fdim
    fa = fdim + 4

    psw = None
    for c0 in range(0, n, CH):
        csz = min(CH, n - c0)
        ngr = csz // 128
        xlT = gat_pool.tile([128, csz], BF16, tag="xlT")
        xrT = gat_pool.tile([128, csz], BF16, tag="xrT")
        eo = gat_pool.tile([2, csz], BF16, tag="eo")
        dr_sb = gat_pool.tile([128, ngr], BF16, tag="dr")
        nc.sync.dma_start(out=xlT[:], in_=xlT_d[:, c0 : c0 + csz])
        nc.sync.dma_start(out=xrT[:], in_=xrT_d[:, c0 : c0 + csz])
        nc.sync.dma_start(out=eo[:], in_=eo_d[:, c0 : c0 + csz])
        nc.sync.dma_start(out=dr_sb[:], in_=dr_d[:, c0 // 128 : c0 // 128 + ngr])

        for b0 in range(0, ngr, bgrp):
            nb = min(bgrp, ngr - b0)
            sps = ps_pool.tile([128, 512], F32, tag="sps")
            pxl = ps_pool.tile([128, 512], F32, tag="pxl")
            for j in range(b0, b0 + nb):
                col = (j - b0) * fdim
                e0 = j * 128
                o = sps[:, col : col + fdim]
                nc.tensor.matmul(out=o, lhsT=xlT[:, e0 : e0 + 128], rhs=wl_sb[:],
                                 start=True, stop=False)
                nc.tensor.matmul(out=o, lhsT=xrT[:, e0 : e0 + 128], rhs=wr_sb[:],
                                 start=False, stop=False)
                nc.tensor.matmul(out=o, lhsT=eo[:, e0 : e0 + 128], rhs=eo_s_sb[:],
                                 start=False, stop=True)
                nc.tensor.matmul(out=pxl[:, col : col + fdim],
                                 lhsT=xlT[:, e0 : e0 + 128], rhs=wl_sb[:],
                                 start=True, stop=True)
            nf = nb * fdim
            r_sb = gat_pool.tile([128, 512], BF16, tag="r")
            nc.scalar.mul(r_sb[:, :nf], sps[:, :nf], NEG_SLOPE)
            m_sb = gat_pool.tile([128, 512], BF16, tag="m")
            nc.vector.tensor_tensor(
                out=m_sb[:, :nf], in0=sps[:, :nf], in1=r_sb[:, :nf],
                op=mybir.AluOpType.max,
            )
            am = gat_pool.tile([128, 512], BF16, tag="am")
            nc.vector.tensor_tensor(
                out=am[:, :nf], in0=m_sb[:, :nf], in1=att_sb[:, :nf],
                op=mybir.AluOpType.mult,
            )
            lgt = gat_pool.tile([128, bgrp * h], F32, tag="lgt")
            nc.vector.tensor_reduce(
                out=lgt[:, : nb * h],
                in_=am[:, :nf].rearrange("p (x c) -> p x c", c=c),
                axis=mybir.AxisListType.X, op=mybir.AluOpType.add,
            )
            ex = gat_pool.tile([128, bgrp * h], BF16, tag="ex")
            nc.scalar.activation(
                ex[:, : nb * h], lgt[:, : nb * h],
                mybir.ActivationFunctionType.Exp,
            )
            aug = gat_pool.tile([128, bgrp * fdim], BF16, tag="aug")
            nc.vector.tensor_tensor(
                out=aug[:, : nb * fdim].rearrange(
                    "p (b hh cc) -> p b hh cc", hh=h, cc=c),
                in0=pxl[:, :nf].rearrange("p (b hh cc) -> p b hh cc", hh=h, cc=c),
                in1=ex[:, : nb * h].rearrange("p (b hh) -> p b hh", hh=h)
                .unsqueeze(3).to_broadcast([128, nb, h, c]),
                op=mybir.AluOpType.mult,
            )
            sel = gat_pool.tile([128, bgrp * 128], BF16, tag="sel")
            nc.vector.tensor_tensor(
                out=sel[:, : nb * 128].rearrange("p (b q) -> p b q", q=128),
                in0=dr_sb[:, b0 : b0 + nb].unsqueeze(2)
                .to_broadcast([128, nb, 128]),
                in1=iota_sb[:].unsqueeze(1).to_broadcast([128, nb, 128]),
                op=mybir.AluOpType.is_equal,
            )
            for j in range(b0, b0 + nb):
                grp = c0 // 128 + j
                wi, jw = divmod(grp, g)
                if jw == 0:
                    psw = win_pool.tile([128, fa], F32, tag="psw")
                jb = j - b0
                nc.tensor.matmul(
                    out=psw[:, :fdim],
                    lhsT=sel[:, jb * 128 : (jb + 1) * 128],
                    rhs=aug[:, jb * fdim : (jb + 1) * fdim],
                    start=(jw == 0), stop=False,
                    skip_group_check=True,
                )
                nc.tensor.matmul(
                    out=psw[:, fdim:fa],
                    lhsT=sel[:, jb * 128 : (jb + 1) * 128],
                    rhs=ex[:, jb * h : (jb + 1) * h],
                    start=False, stop=(jw == g - 1),
                    skip_group_check=True,
                )
                if jw == g - 1:
                    rows = min(WIN, vd - WIN * wi)
                    osb = gat_pool.tile([128, fa], F32, tag="osb")
                    nc.scalar.copy(osb[:], psw[:])
                    nc.sync.dma_start(
                        out=slab[WIN * wi : WIN * wi + rows, :], in_=osb[:rows],
                    )


def _emit_sl(nc, pools, wsl_sb, xT_sb, out_d, cx, v):
    """out_d[cx, v] f32 = wsl.T @ x  (self-loop linear), chunked by 512."""
    sb_pool, ps_pool = pools
    for i in range(math.ceil(v / 512)):
        m = min(512, v - i * 512)
        ps = ps_pool.tile([cx, 512], F32, tag="slps")
        nc.tensor.matmul(
            out=ps[:, :m], lhsT=wsl_sb[:], rhs=xT_sb[:, i * 512 : i * 512 + m],
            start=True, stop=True,
        )
        sb = sb_pool.tile([cx, 512], F32, tag="slsb")
        nc.scalar.copy(sb[:, :m], ps[:, :m])
        nc.sync.dma_start(out=out_d[:, i * 512 : i * 512 + m], in_=sb[:, :m])


def _load_sb(nc, pool, name, arr_shape, dtype, dram):
    t = pool.tile(list(arr_shape), dtype, tag=name)
    nc.sync.dma_start(out=t[:], in_=dram[:])
    return t


def _build_launch1(cfg: Cfg, g_sg, g_gs, w_sg, w_gs, parts=("sl", "sg", "gs")):
    n_sg = w_sg * g_sg * 128
    n_gs = w_gs * g_gs * 128
    hc1 = cfg.hc1
    nc = bacc.Bacc("TRN2", target_bir_lowering=False, debug=False,
                   num_devices=cfg.ncores)
    dI, dO = "ExternalInput", "ExternalOutput"
    D = {}
    def di(name, shape, dt):
        D[name] = nc.dram_tensor(name, list(shape), dt, kind=dI)
        return D[name]

    di("xmT", (128, cfg.n_s), BF16)
    di("iota", (128, 128), BF16)
    for r in ("sg", "gs"):
        di(f"w_{r}_l", (cfg.d, hc1), BF16)
        di(f"w_{r}_r", (cfg.d, hc1), BF16)
        di(f"eo_{r}_rhs", (2, hc1), BF16)
        di(f"att_{r}", (128, 512), BF16)
    di("wsl1", (cfg.d, cfg.c1), BF16)
    for r, n in (("sg", n_sg), ("gs", n_gs)):
        di(f"{r}_xlT", (128, n), BF16)
        di(f"{r}_xrT", (128, n), BF16)
        di(f"{r}_eo", (2, n), BF16)
        di(f"{r}_dr", (128, n // 128), BF16)

    sg_slab = nc.dram_tensor("sg_slab", [cfg.gsh, hc1 + 4], F32, kind=dO)
    gs_slab = nc.dram_tensor("gs_slab", [cfg.n_s, hc1 + 4], F32, kind=dO)
    sl1 = nc.dram_tensor("sl1", [cfg.c1, cfg.n_s], F32, kind=dO)

    with tile.TileContext(nc) as tc:
        import contextlib
        with contextlib.ExitStack() as ctx:
            persist = ctx.enter_context(tc.tile_pool(name="persist", bufs=1))
            sb_pool = ctx.enter_context(tc.tile_pool(name="sb", bufs=3))
            ps_pool = ctx.enter_context(
                tc.tile_pool(name="ps", bufs=2, space="PSUM"))
            win_pool = ctx.enter_context(
                tc.tile_pool(name="win", bufs=2, space="PSUM"))
            gat_pool = ctx.enter_context(tc.tile_pool(name="gat", bufs=2))

            S = {}
            for name in ["xmT", "iota", "wsl1", "w_sg_l", "w_sg_r",
                         "eo_sg_rhs", "att_sg", "w_gs_l", "w_gs_r",
                         "eo_gs_rhs", "att_gs"]:
                d = D[name]
                S[name] = _load_sb(nc, persist, name, d.shape, d.dtype, d)

            tp = (sb_pool, ps_pool)
            if "sl" in parts:
                _emit_sl(nc, tp, S["wsl1"], S["xmT"], sl1, cfg.c1, cfg.n_s)

            pools = (sb_pool, ps_pool, gat_pool, win_pool)
            if "sg" in parts:
                _emit_pass(nc, tc, pools, (S["iota"], S["att_sg"]), n=n_sg,
                           w_count=w_sg, g=g_sg, fdim=hc1, h=cfg.h,
                           xlT_d=D["sg_xlT"], xrT_d=D["sg_xrT"],
                           eo_d=D["sg_eo"], dr_d=D["sg_dr"],
                           wl_sb=S["w_sg_l"], wr_sb=S["w_sg_r"],
                           eo_s_sb=S["eo_sg_rhs"], slab=sg_slab, vd=cfg.gsh)
            if "gs" in parts:
                _emit_pass(nc, tc, pools, (S["iota"], S["att_gs"]), n=n_gs,
                           w_count=w_gs, g=g_gs, fdim=hc1, h=cfg.h,
                           xlT_d=D["gs_xlT"], xrT_d=D["gs_xrT"],
                           eo_d=D["gs_eo"], dr_d=D["gs_dr"],
                           wl_sb=S["w_gs_l"], wr_sb=S["w_gs_r"],
                           eo_s_sb=S["eo_gs_rhs"], slab=gs_slab, vd=cfg.n_s)
    nc.compile()
    return nc


def _build_launch2(cfg: Cfg, g_gs, w_gs):
    n_gs = w_gs * g_gs * 128
    hc1, hc3 = cfg.hc1, cfg.hc3
    nc = bacc.Bacc("TRN2", target_bir_lowering=False, debug=False,
                   num_devices=cfg.ncores)
    dI, dO = "ExternalInput", "ExternalOutput"
    D = {}
    def di(name, shape, dt):
        D[name] = nc.dram_tensor(name, list(shape), dt, kind=dI)
        return D[name]

    di("x1mT", (128, cfg.n_s), BF16)
    di("iota", (128, 128), BF16)
    di("w3_l", (hc1, hc3), BF16)
    di("w3_r", (hc1, hc3), BF16)
    di("eo3_rhs", (2, hc3), BF16)
    di("att3", (128, 512), BF16)
    di("wsl3", (hc1, cfg.c3), BF16)
    di("l3_xlT", (128, n_gs), BF16)
    di("l3_xrT", (128, n_gs), BF16)
    di("l3_eo", (2, n_gs), BF16)
    di("l3_dr", (128, n_gs // 128), BF16)

    l3_slab = nc.dram_tensor("l3_slab", [cfg.n_s, hc3 + 4], F32, kind=dO)
    sl3 = nc.dram_tensor("sl3", [cfg.c3, cfg.n_s], F32, kind=dO)

    with tile.TileContext(nc) as tc:
        import contextlib
        with contextlib.ExitStack() as ctx:
            persist = ctx.enter_context(tc.tile_pool(name="persist", bufs=1))
            sb_pool = ctx.enter_context(tc.tile_pool(name="sb", bufs=3))
            ps_pool = ctx.enter_context(
                tc.tile_pool(name="ps", bufs=2, space="PSUM"))
            win_pool = ctx.enter_context(
                tc.tile_pool(name="win", bufs=2, space="PSUM"))
            gat_pool = ctx.enter_context(tc.tile_pool(name="gat", bufs=2))

            S = {}
            for name in ["x1mT", "iota", "w3_l", "w3_r", "eo3_rhs", "att3",
                         "wsl3"]:
                d = D[name]
                S[name] = _load_sb(nc, persist, name, d.shape, d.dtype, d)

            tp = (sb_pool, ps_pool)
            _emit_sl(nc, tp, S["wsl3"], S["x1mT"], sl3, cfg.c3, cfg.n_s)
            pools = (sb_pool, ps_pool, gat_pool, win_pool)
            _emit_pass(nc, tc, pools, (S["iota"], S["att3"]), n=n_gs,
                       w_count=w_gs, g=g_gs, fdim=hc3, h=cfg.h,
                       xlT_d=D["l3_xlT"], xrT_d=D["l3_xrT"],
                       eo_d=D["l3_eo"], dr_d=D["l3_dr"],
                       wl_sb=S["w3_l"], wr_sb=S["w3_r"],
                       eo_s_sb=S["eo3_rhs"], slab=l3_slab, vd=cfg.n_s)
    nc.compile()
    return nc


def _edge_stage(prefix, xsT, xdT, S, D, Dr, Ea, n):
    """Stage host-gathered raw features + [ea; 1] + dst_rel for one pass."""
    eo = np.zeros((2, n), np.float32)
    eo[0] = Ea
    eo[1] = 1.0
    return {
        f"{prefix}_xlT": bf(xsT[:, S]),
        f"{prefix}_xrT": bf(xdT[:, D]),
        f"{prefix}_eo": bf(eo),
        f"{prefix}_dr": bf(_pm(Dr, n)),
    }


def _host_prep(inputs, cfg: Cfg):
    nco, gsh = cfg.ncores, cfg.gsh
    sg_src = np.asarray(inputs["sg_src"]).astype(np.int64)
    sg_dst = np.asarray(inputs["sg_dst"]).astype(np.int64)
    gs_src = np.asarray(inputs["gs_src"]).astype(np.int64)
    gs_dst = np.asarray(inputs["gs_dst"]).astype(np.int64)
    ea_sg = f32(inputs["ea_sg"]).ravel()
    ea_gs = f32(inputs["ea_gs"]).ravel()

    w_sg = math.ceil(gsh / WIN)
    w_gs = math.ceil(cfg.n_s / WIN)

    per_core = []
    g_sg = g_gs = 1
    for c in range(nco):
        m_sg = (sg_dst // gsh) == c
        m_gs = (gs_src // gsh) == c
        e_sg = (sg_src[m_sg], sg_dst[m_sg] - c * gsh, ea_sg[m_sg])
        e_gs = (gs_src[m_gs] - c * gsh, gs_dst[m_gs], ea_gs[m_gs])
        per_core.append((e_sg, e_gs))
        cnt1 = np.bincount(e_sg[1] // WIN, minlength=w_sg)
        cnt2 = np.bincount(e_gs[1] // WIN, minlength=w_gs)
        g_sg = max(g_sg, math.ceil(cnt1.max() / 128))
        g_gs = max(g_gs, math.ceil(cnt2.max() / 128))
    n_sg = w_sg * g_sg * 128
    n_gs = w_gs * g_gs * 128

    core_edges = []
    for c in range(nco):
        (s1, d1, e1), (s2, d2, e2) = per_core[c]
        sg = _pack_edges(s1, d1, e1, w_sg, g_sg)
        gs = _pack_edges(s2, d2, e2, w_gs, g_gs)
        core_edges.append((sg, gs))

    hc1 = cfg.hc1
    common1 = {
        "xmT": bf(f32(inputs["x_mrna"]).T),
        "iota": bf(np.tile(np.arange(128, dtype=np.float32), (128, 1))),
        "wsl1": bf(inputs["Wsl1"]),
    }
    for r in ("sg", "gs"):
        common1[f"w_{r}_l"] = bf(inputs[f"Wl1_{r}"])
        common1[f"w_{r}_r"] = bf(inputs[f"Wr1_{r}"])
        eo_rhs = np.stack([
            f32(inputs[f"We1_{r}"]).reshape(hc1),
            f32(inputs[f"bl1_{r}"]) + f32(inputs[f"br1_{r}"]),
        ])
        common1[f"eo_{r}_rhs"] = bf(eo_rhs)
        att = f32(inputs[f"att1_{r}"]).reshape(hc1)
        common1[f"att_{r}"] = bf(np.tile(np.tile(att, 512 // hc1), (128, 1)))

    xmT = f32(inputs["x_mrna"]).T
    xgT_full = f32(inputs["x_gene"]).T
    in_maps1 = []
    for c in range(nco):
        sg, gs = core_edges[c]
        xgT = np.ascontiguousarray(xgT_full[:, c * gsh : (c + 1) * gsh])
        m = dict(common1)
        m.update(_edge_stage("sg", xmT, xgT, *sg, n_sg))
        m.update(_edge_stage("gs", xgT, xmT, *gs, n_gs))
        in_maps1.append(m)

    return {
        "w_sg": w_sg, "w_gs": w_gs, "g_sg": g_sg, "g_gs": g_gs,
        "n_sg": n_sg, "n_gs": n_gs, "in_maps1": in_maps1,
        "core_edges": core_edges,
    }


def _host_mid(cfg: Cfg, res1, meta, inputs):
    h, c1, hc1 = cfg.h, cfg.c1, cfg.hc1

    def combine(ws, dn, blbo, bo):
        dnr = np.repeat(dn, c1, axis=1)
        return np.where(dnr > 0,
                        ws / np.where(dnr > 0, dnr, 1.0) + blbo.reshape(1, hc1),
                        bo.reshape(1, hc1))

    bo_sg = f32(inputs["bo1_sg"])
    blbo_sg = f32(inputs["bl1_sg"]) + bo_sg
    xg_parts = []
    for c in range(cfg.ncores):
        slab = res1.results[c]["sg_slab"]
        xg_parts.append(np.maximum(
            combine(slab[:, :hc1], slab[:, hc1:], blbo_sg, bo_sg), 0.0))
    x1_gene = np.concatenate(xg_parts, axis=0)

    ws = sum(res1.results[c]["gs_slab"][:, :hc1] for c in range(cfg.ncores))
    dn = sum(res1.results[c]["gs_slab"][:, hc1:] for c in range(cfg.ncores))
    bo_gs = f32(inputs["bo1_gs"])
    blbo_gs = f32(inputs["bl1_gs"]) + bo_gs
    sl1 = res1.results[0]["sl1"].T + f32(inputs["bsl1"])  # [n_s, c1]
    x1_mrna = np.maximum(
        combine(ws, dn, blbo_gs, bo_gs) + np.tile(sl1, (1, h)), 0.0)
    return x1_gene, x1_mrna


def _host_prep2(cfg: Cfg, inputs, meta, x1_gene, x1_mrna):
    hc1, hc3 = cfg.hc1, cfg.hc3
    eo3 = np.stack([
        f32(inputs["We3_gs"]).reshape(hc3),
        f32(inputs["bl3_gs"]) + f32(inputs["br3_gs"]),
    ])
    common = {
        "x1mT": bf(x1_mrna.T),
        "iota": bf(np.tile(np.arange(128, dtype=np.float32), (128, 1))),
        "w3_l": bf(inputs["Wl3_gs"]),
        "w3_r": bf(inputs["Wr3_gs"]),
        "eo3_rhs": bf(eo3),
        "att3": bf(np.tile(np.tile(f32(inputs["att3_gs"]).reshape(hc3),
                                   512 // hc3), (128, 1))),
        "wsl3": bf(inputs["Wsl3"]),
    }
    x1gT_full = x1_gene.T
    x1mT = x1_mrna.T
    in_maps2 = []
    for c in range(cfg.ncores):
        _, gs = meta["core_edges"][c]
        x1gT = np.ascontiguousarray(
            x1gT_full[:, c * cfg.gsh : (c + 1) * cfg.gsh])
        m = dict(common)
        m.update(_edge_stage("l3", x1gT, x1mT, *gs, meta["n_gs"]))
        in_maps2.append(m)
    return in_maps2


def _host_final(cfg: Cfg, res2, inputs):
    hc3, c3 = cfg.hc3, cfg.c3
    ws = sum(res2.results[c]["l3_slab"][:, :hc3] for c in range(cfg.ncores))
    dn = sum(res2.results[c]["l3_slab"][:, hc3:] for c in range(cfg.ncores))
    sl3 = res2.results[0]["sl3"].T + f32(inputs["bsl3"])  # [n_s, c3]
    bo3 = f32(inputs["bo3_gs"]).reshape(1, 1, c3)
    blbo3 = (f32(inputs["bl3_gs"]) + np.tile(f32(inputs["bo3_gs"]), cfg.h)
             ).reshape(1, cfg.h, c3)
    dn3 = dn[:, :, None]
    out = np.where(dn3 > 0,
                   ws.reshape(cfg.n_s, cfg.h, c3) / np.where(dn3 > 0, dn3, 1.0)
                   + blbo3,
                   bo3)
    return np.maximum(out.mean(axis=1) + sl3, 0.0).astype(np.float32)


_trace = [False]


def kernel(**inputs):
    cfg = Cfg()
    meta = _host_prep(inputs, cfg)
    nc1 = _build_launch1(cfg, meta["g_sg"], meta["g_gs"], meta["w_sg"],
                         meta["w_gs"])
    cores = list(range(cfg.ncores))
    res1 = run_bass_kernel_spmd(nc1, meta["in_maps1"], cores,
                                trace=_trace[0])
    x1_gene, x1_mrna = _host_mid(cfg, res1, meta, inputs)
    in_maps2 = _host_prep2(cfg, inputs, meta, x1_gene, x1_mrna)
    nc2 = _build_launch2(cfg, meta["g_gs"], meta["w_gs"])
    res2 = run_bass_kernel_spmd(nc2, in_maps2, cores, trace=_trace[0])
    out = _host_final(cfg, res2, inputs)
    kernel.last = {"res1": res1, "res2": res2, "x1_gene": x1_gene,
                   "x1_mrna": x1_mrna, "meta": meta}
    return out
